# revision 1
# baseline (speedup 1.0000x reference)
"""nn_Head single-head causal attention on 8 TRN2 NeuronCores.

Full inputs: x [8, 2048, 1024] f32, Wk/Wq/Wv [1024, 64] f32.
Full output: [8, 2048, 64] f32 = softmax(causal(q k^T * C^-0.5)) @ v per batch.

Sharding: data-parallel over batch B=8 -> one batch element per core;
weights replicated. No collectives.

Per-core kernel (Bass/Tile, f32r matmuls + bf16 probability/value stage):
  A) load x t-tiles, PE-transpose to xT [c-part, t] (TensorE contracts over
     the partition dim, so x must be c-major; fp32 cannot DMA-transpose)
  B) QKV: kT/qT/vT [h(64), t] via lhsT=W [c,64], rhs=xT; v transposed back
     to natural v1 [s-part, t-tile, H+1] bf16 with a ones column at H that
     makes the PV matmul also produce the softmax denominator
  C) S^T tile = kT_slice^T@qT -> PSUM [s 128, t 512]; exp on ScalarE with
     scale=C^-0.5 folded in (scores are O(1): no max-subtraction needed,
     mathematically identical softmax); causality via memset of fully-masked
     column ranges + a 0/1 upper-triangular mask mul on diagonal tiles;
     PV: po[h|denom, t] += v1_slice^T @ P^T accumulated over s-tiles
  E) po -> SBUF, PE-transpose to [t-part, H+1], multiply by per-partition
     reciprocal of the denominator column, single output DMA.
"""

from contextlib import ExitStack

import numpy as np

import concourse.bass as bass
import concourse.mybir as mybir
import concourse.tile as tile
from concourse import bass_utils
from concourse.masks import make_identity

B, T, C, H = 8, 2048, 1024, 64
N_CORES = 8
P = 128


def _patch_drain_split():
    """This walrus build accepts only one sem wait per instruction ("Too many
    sync wait commands" in setupSyncWait otherwise). Hoist extra waits onto
    same-engine NOPs ahead of the instruction (engine streams dispatch
    in-order, so the blocking semantics are identical), and split the
    TileContext tail drain the same way."""
    if getattr(tile.TileContext, "_drain_split_patched", False):
        return
    from concourse.tile import ScopedClock

    _orig_add = tile.TileContext._add_instruction

    def _patched_add(self, inst):
        si = getattr(inst, "sync_info", None)
        if si is not None and si.on_wait and len(si.on_wait) > 1:
            waits = list(si.on_wait)
            for i, w in enumerate(waits[:-1]):
                nop = mybir.InstNoOp(
                    name=f"{inst.name}-ws{i}",
                    sync_info=mybir.SyncInfo(on_wait=[w], on_update=[]),
                    bass_nofuse=True,
                    engine=inst.engine,
                )
                _orig_add(self, nop)
            si.on_wait = waits[-1:]
            inst.sync_info = si
        _orig_add(self, inst)

    tile.TileContext._add_instruction = _patched_add

    def _patched_dab(self, tick_clock, wait_clock):
        nc = self.nc
        drain_inst = nc.sync.drain()
        wait_clock.add_sem_waits(
            drain_inst.ins, ScopedClock({None: tick_clock.global_clock})
        )
        si = drain_inst.ins.sync_info
        if si is not None and si.on_wait and len(si.on_wait) > 1:
            waits = list(si.on_wait)
            si.on_wait = waits[:1]
            drain_inst.ins.sync_info = si
            for w in waits[1:]:
                d2 = nc.sync.drain()
                d2.ins.sync_info = mybir.SyncInfo(on_wait=[w], on_update=[])
        nc.all_engine_barrier()
        popped = nc._tile_sem_poison_stack.pop()
        assert popped is self._sem_poison
        nc.clear_and_free_semaphores(list(self.sems.allocated().values()))
        nc.all_engine_barrier()

    tile.TileContext._drain_and_barrier = _patched_dab
    tile.TileContext._drain_split_patched = True


def _emit(tc, out_d, x_d, wk_d, wq_d, wv_d):
    nc = tc.nc
    f32r = mybir.dt.float32r
    f32 = mybir.dt.float32
    bf16 = mybir.dt.bfloat16
    Exp = mybir.ActivationFunctionType.Exp

    CT = C // P  # 8 c-tiles
    TT = T // P  # 16 t-tiles
    BLK = 512
    NB = T // BLK  # 4 t-blocks
    SPB = BLK // P  # 4 s-tiles per block width
    H1 = H + 1
    scale = float(C) ** -0.5

    with ExitStack() as ctx:
        const = ctx.enter_context(tc.tile_pool(name="const", bufs=1))
        persist = ctx.enter_context(tc.tile_pool(name="persist", bufs=1))
        xa_pool = ctx.enter_context(tc.tile_pool(name="xa", bufs=6))
        pt_pool = ctx.enter_context(tc.tile_pool(name="ptp", bufs=4))
        oT_pool = ctx.enter_context(tc.tile_pool(name="otp", bufs=2))
        rec_pool = ctx.enter_context(tc.tile_pool(name="recp", bufs=2))
        # PSUM: 8 banks total so all phases can overlap.
        psA = ctx.enter_context(tc.tile_pool(name="psA", bufs=1, space="PSUM"))
        psB = ctx.enter_context(tc.tile_pool(name="psB", bufs=2, space="PSUM"))
        psS = ctx.enter_context(tc.tile_pool(name="psS", bufs=2, space="PSUM"))
        psOE = ctx.enter_context(tc.tile_pool(name="psOE", bufs=1, space="PSUM"))

        # identity: build in f32 (memset on f32r is invalid ISA in this
        # walrus), keep an f32r copy for same-dtype transposes
        ident = const.tile([P, P], f32, name="ident")
        make_identity(nc, ident)
        identr = const.tile([P, P], f32r, name="identr")
        nc.vector.tensor_copy(out=identr, in_=ident)
        # 0/1 mask: mask[s, t] = 1 iff s <= t (keep causal entries)
        mask = const.tile([P, P], bf16, name="mask")
        nc.vector.memset(mask, 1.0)
        nc.gpsimd.affine_select(
            out=mask,
            in_=mask,
            compare_op=mybir.AluOpType.is_ge,
            fill=0.0,
            base=0,
            pattern=[[1, P]],
            channel_multiplier=-1,
        )

        # [Wk | Wq] packed: one M=128 matmul produces k on partitions 0-63
        # and q on 64-127
        wkq_sb = const.tile([P, CT, 2 * H], f32r, name="wkq_sb")
        wv_sb = const.tile([P, CT, H], f32r, name="wv_sb")

        xT = persist.tile([P, CT, T], f32r, name="xT")
        kT = persist.tile([H, T], f32r, name="kT")
        qT = persist.tile([H, T], f32r, name="qT")
        vT = persist.tile([H, T], f32, name="vT")
        v1 = persist.tile([P, TT, H1], bf16, name="v1")
        out_sb = persist.tile([P, TT, H], f32, name="out_sb")

        nc.vector.memset(v1[:, :, H : H + 1], 1.0)

        # Phase A: x -> xT via PE transpose. Weight DMAs are emitted after
        # the first x tiles so they don't delay the transpose pipeline.
        for tt in range(TT):
            tsl = slice(tt * P, (tt + 1) * P)
            xa = xa_pool.tile([P, C], f32r, name="xa")
            nc.sync.dma_start(xa, x_d[tsl, :])
            if tt == 3:
                nc.sync.dma_start(
                    wkq_sb[:, :, 0:H], wk_d.rearrange("(o p) h -> p o h", p=P)
                )
                nc.sync.dma_start(
                    wkq_sb[:, :, H : 2 * H],
                    wq_d.rearrange("(o p) h -> p o h", p=P),
                )
                nc.sync.dma_start(
                    wv_sb, wv_d.rearrange("(o p) h -> p o h", p=P)
                )
            for cg in range(CT // 4):
                ps_t = psA.tile([P, 4, P], f32r, name="ps_t")
                for j in range(4):
                    ci = cg * 4 + j
                    nc.tensor.transpose(
                        ps_t[:, j, :], xa[:, ci * P : (ci + 1) * P], identr
                    )
                dst = xT[:, cg * 4 : cg * 4 + 4, tsl]
                if (tt + cg) % 2 == 0:
                    nc.vector.tensor_copy(out=dst, in_=ps_t)
                else:
                    nc.scalar.copy(out=dst, in_=ps_t)

        # Phase B: QKV projections (kq packed) + v back to natural layout
        for bi in range(NB):
            tsl = slice(bi * BLK, (bi + 1) * BLK)
            pkq = psB.tile([P, BLK], f32, name="pkq", tag="qkv")
            for ci in range(CT):
                nc.tensor.matmul(
                    pkq,
                    wkq_sb[:, ci, :],
                    xT[:, ci, tsl],
                    start=(ci == 0),
                    stop=(ci == CT - 1),
                )
            nc.vector.tensor_copy(out=kT[:, tsl], in_=pkq[0:H, :])
            # partition-shift copy 64-127 -> 0-63 (legal on DVE)
            nc.vector.tensor_copy(out=qT[:, tsl], in_=pkq[H:P, :])
            pv = psB.tile([H, BLK], f32, name="pv", tag="qkv")
            for ci in range(CT):
                nc.tensor.matmul(
                    pv,
                    wv_sb[:, ci, :],
                    xT[:, ci, tsl],
                    start=(ci == 0),
                    stop=(ci == CT - 1),
                )
            nc.vector.tensor_copy(out=vT[:, tsl], in_=pv)
            for c4 in range(SPB):
                st = bi * SPB + c4
                pvt = psB.tile([P, H], f32, name="pvt", tag="qkv")
                nc.tensor.transpose(
                    pvt, vT[:, st * P : (st + 1) * P], ident[:H, :H]
                )
                nc.vector.tensor_copy(out=v1[:, st, 0:H], in_=pvt)

        # Phase C: attention
        for bi in range(NB):
            tsl = slice(bi * BLK, (bi + 1) * BLK)
            po = psOE.tile([H1, BLK], f32, name="po", tag="poe")
            NS = SPB * (bi + 1)
            for g in range(NS // 2):
                ps_s = psS.tile([P, 2, BLK], f32, name="ps_s")
                for j in range(2):
                    st = 2 * g + j
                    nc.tensor.matmul(
                        ps_s[:, j, :],
                        kT[:, st * P : (st + 1) * P],
                        qT[:, tsl],
                        start=True,
                        stop=True,
                    )
                ptile = pt_pool.tile([P, 2, BLK], bf16, name="ptile")
                d0s = [max(0, (2 * g + j) * P - bi * BLK) for j in range(2)]
                if d0s[0] == 0 and d0s[1] == 0:
                    nc.scalar.activation(ptile, ps_s, Exp, scale=scale)
                else:
                    # skip fully-masked prefix columns: exp only the valid
                    # suffix, zero the prefix on DVE
                    for j in range(2):
                        d0 = d0s[j]
                        nc.scalar.activation(
                            ptile[:, j, d0:], ps_s[:, j, d0:], Exp, scale=scale
                        )
                        if d0 > 0:
                            nc.vector.memset(ptile[:, j, 0:d0], 0.0)
                for j in range(2):
                    st = 2 * g + j
                    d0 = st * P - bi * BLK
                    if d0 >= 0:  # tile touches/precedes the diagonal
                        nc.vector.tensor_mul(
                            ptile[:, j, d0 : d0 + P],
                            ptile[:, j, d0 : d0 + P],
                            mask,
                        )
                for j in range(2):
                    st = 2 * g + j
                    nc.tensor.matmul(
                        po,
                        v1[:, st, 0:H1],
                        ptile[:, j, :],
                        start=(st == 0),
                        stop=(st == NS - 1),
                    )

            oT = oT_pool.tile([H1, BLK], f32, name="oT")
            nc.vector.tensor_copy(out=oT, in_=po)
            for c4 in range(SPB):
                pe = psOE.tile([P, H1], f32, name="pe", tag="poe")
                nc.tensor.transpose(
                    pe, oT[:, c4 * P : (c4 + 1) * P], ident[:H1, :H1]
                )
                rec = rec_pool.tile([P, 1], f32, name="rec")
                nc.vector.reciprocal(rec, pe[:, H:H1])
                nc.vector.tensor_scalar_mul(
                    out_sb[:, bi * SPB + c4, :], pe[:, 0:H], rec
                )
            # stream this block's rows out while later blocks compute
            nc.sync.dma_start(
                out_d.rearrange("(o p) h -> p o h", p=P)[
                    :, bi * SPB : (bi + 1) * SPB, :
                ],
                out_sb[:, bi * SPB : (bi + 1) * SPB, :],
            )


_NC_CACHE = {}


def build_nc():
    if "nc" in _NC_CACHE:
        return _NC_CACHE["nc"]
    _patch_drain_split()
    f32r = mybir.dt.float32r
    f32 = mybir.dt.float32
    nc = bass.Bass(
        "TRN2", target_bir_lowering=False, debug=False, num_devices=N_CORES
    )
    x_d = nc.dram_tensor("x", [T, C], f32r, kind="ExternalInput").ap()
    wk_d = nc.dram_tensor("Wk", [C, H], f32r, kind="ExternalInput").ap()
    wq_d = nc.dram_tensor("Wq", [C, H], f32r, kind="ExternalInput").ap()
    wv_d = nc.dram_tensor("Wv", [C, H], f32r, kind="ExternalInput").ap()
    out_d = nc.dram_tensor("out", [T, H], f32, kind="ExternalOutput").ap()
    with tile.TileContext(nc) as tc:
        _emit(tc, out_d, x_d, wk_d, wq_d, wv_d)
    _NC_CACHE["nc"] = nc
    return nc


def kernel(x, Wk, Wq, Wv, **run_kwargs):
    """Full-input entry point: shard over batch, run on cores 0-7, gather."""
    x = np.ascontiguousarray(np.asarray(x), dtype=np.float32)
    Wk = np.ascontiguousarray(np.asarray(Wk), dtype=np.float32)
    Wq = np.ascontiguousarray(np.asarray(Wq), dtype=np.float32)
    Wv = np.ascontiguousarray(np.asarray(Wv), dtype=np.float32)
    assert x.shape == (B, T, C), x.shape

    nc = build_nc()
    in_maps = [
        {"x": np.ascontiguousarray(x[b]), "Wk": Wk, "Wq": Wq, "Wv": Wv}
        for b in range(B)
    ]
    res = bass_utils.run_bass_kernel_spmd(
        nc, in_maps, core_ids=list(range(N_CORES)), **run_kwargs
    )
    out = np.stack([res.results[b]["out"] for b in range(B)], axis=0)
    if run_kwargs:
        kernel.last_results = res
    return out.astype(np.float32)



# revision 20
# speedup vs baseline: 1.1631x; 1.1631x over previous
"""nn_Head single-head causal attention on 8 TRN2 NeuronCores.

Full inputs: x [8, 2048, 1024] f32, Wk/Wq/Wv [1024, 64] f32.
Full output: [8, 2048, 64] f32 = softmax(causal(q k^T * C^-0.5)) @ v per batch.

Sharding: data-parallel over batch B=8 -> one batch element per core;
weights replicated. No collectives.

Per-core kernel (Bass/Tile), pipelined behind the x DMA stream in 8 chunks
of 2 t-tiles each:
  A) per chunk: PE-transpose x (f32r) -> PSUM, copy-cast to xT bf16
     [c-part, t] (DVE/ACT split). Weights cast to bf16 once (Pool).
  B) per chunk: v in natural [t-part, h] layout via bf16 matmuls
     (lhsT = xT tile, rhs = Wv); per 2 chunks: packed [Wk|Wq] matmul
     -> kT/qT bf16 [h, t].
  C) attention per 512-col t-block with exact 128-granular causal
     trimming: S^T tile = kT_slice^T @ qT into PSUM (suffix-only on
     diagonal tiles); causal masking via an extra identity^T @ step
     matmul accumulating -30000 into the masked triangle (exp -> 0);
     exp on ScalarE with scale=C^-0.5 folded in; PV accumulates
     po[h|denom, t] with a ones-column in v1 producing the softmax
     denominator.
  D) out stage per block: po -> SBUF, PE-transpose, batched reciprocal
     of the denominator column, per-partition scalar-mul into out_sb,
     streaming output DMA.
"""

from contextlib import ExitStack

import numpy as np

import concourse.bass as bass
import concourse.mybir as mybir
import concourse.tile as tile
from concourse import bass_utils
from concourse.masks import make_identity

B, T, C, H = 8, 2048, 1024, 64
N_CORES = 8
P = 128


def _patch_drain_split():
    """This walrus build accepts only one sem wait per instruction ("Too many
    sync wait commands" in setupSyncWait otherwise). Hoist extra waits onto
    same-engine NOPs ahead of the instruction (engine streams dispatch
    in-order, so the blocking semantics are identical), and split the
    TileContext tail drain the same way."""
    if getattr(tile.TileContext, "_drain_split_patched", False):
        return
    from concourse.tile import ScopedClock

    _orig_add = tile.TileContext._add_instruction

    def _patched_add(self, inst):
        si = getattr(inst, "sync_info", None)
        if si is not None and si.on_wait and len(si.on_wait) > 1:
            waits = list(si.on_wait)
            for i, w in enumerate(waits[:-1]):
                nop = mybir.InstNoOp(
                    name=f"{inst.name}-ws{i}",
                    sync_info=mybir.SyncInfo(on_wait=[w], on_update=[]),
                    bass_nofuse=True,
                    engine=inst.engine,
                )
                _orig_add(self, nop)
            si.on_wait = waits[-1:]
            inst.sync_info = si
        _orig_add(self, inst)

    tile.TileContext._add_instruction = _patched_add

    def _patched_dab(self, tick_clock, wait_clock):
        nc = self.nc
        drain_inst = nc.sync.drain()
        wait_clock.add_sem_waits(
            drain_inst.ins, ScopedClock({None: tick_clock.global_clock})
        )
        si = drain_inst.ins.sync_info
        if si is not None and si.on_wait and len(si.on_wait) > 1:
            waits = list(si.on_wait)
            si.on_wait = waits[:1]
            drain_inst.ins.sync_info = si
            for w in waits[1:]:
                d2 = nc.sync.drain()
                d2.ins.sync_info = mybir.SyncInfo(on_wait=[w], on_update=[])
        nc.all_engine_barrier()
        popped = nc._tile_sem_poison_stack.pop()
        assert popped is self._sem_poison
        nc.clear_and_free_semaphores(list(self.sems.allocated().values()))
        nc.all_engine_barrier()

    tile.TileContext._drain_and_barrier = _patched_dab
    tile.TileContext._drain_split_patched = True


EMIT_LOG = []


def _emit(tc, out_d, x_d, wk_d, wq_d, wv_d):
    nc = tc.nc
    EMIT_LOG.clear()

    def mark(label):
        n = nc.next_id()  # consumes one id; fine for attribution
        EMIT_LOG.append((n, label))

    f32r = mybir.dt.float32r
    f32 = mybir.dt.float32
    bf16 = mybir.dt.bfloat16
    Exp = mybir.ActivationFunctionType.Exp

    CT = C // P  # 8 c-tiles
    TT = T // P  # 16 t-tiles
    CW = 256  # attention block width in t-columns (2 t-tiles)
    NBLK = T // CW  # 8 blocks
    H1 = H + 1
    scale = float(C) ** -0.5
    BIG = 30000.0  # scale*BIG ~ 937 >> 88: exp underflows to exactly 0
    # attention t-blocks = 256-col chunks: block k spans chunk k's columns;
    # items are PAIRS of s-tiles sharing one PSUM tile and one exp

    x_r = x_d.rearrange("(k p) c -> k p c", k=TT, p=P)
    out_r = out_d.rearrange("(o p) h -> p o h", p=P)

    with ExitStack() as ctx:
        const = ctx.enter_context(tc.tile_pool(name="const", bufs=1))
        persist = ctx.enter_context(tc.tile_pool(name="persist", bufs=1))
        xa_pool = ctx.enter_context(tc.tile_pool(name="xa", bufs=5))
        pt_pool = ctx.enter_context(tc.tile_pool(name="ptp", bufs=4))
        oT_pool = ctx.enter_context(tc.tile_pool(name="otp", bufs=2))
        rec_pool = ctx.enter_context(tc.tile_pool(name="recp", bufs=2))
        # PSUM: 8 banks. psA (2) hosts transpose + v-nat tiles, psB (2)
        # the kq + out-stage transpose tiles, psO (2) the per-block po
        # accumulators, psS (2) the S tiles.
        psA = ctx.enter_context(tc.tile_pool(name="psA", bufs=2, space="PSUM"))
        psB = ctx.enter_context(tc.tile_pool(name="psB", bufs=2, space="PSUM"))
        psO = ctx.enter_context(tc.tile_pool(name="psO", bufs=2, space="PSUM"))
        psS = ctx.enter_context(tc.tile_pool(name="psS", bufs=2, space="PSUM"))

        # identity in f32 (memset on f32r is invalid ISA in this walrus),
        # plus f32r and bf16 copies for same-dtype transposes / mask matmul
        ident = const.tile([P, P], f32, name="ident")
        make_identity(nc, ident)
        identr = const.tile([P, P], f32r, name="identr")
        nc.vector.tensor_copy(out=identr, in_=ident)
        identb = const.tile([P, P], bf16, name="identb")
        nc.vector.tensor_copy(out=identb, in_=ident)
        # step[p, c] = -BIG iff p > c else 0: ident^T @ step accumulated into
        # a diagonal S tile adds -BIG above the diagonal -> exp gives 0
        stepb = const.tile([P, P], bf16, name="stepb")
        nc.vector.memset(stepb, -BIG)
        nc.gpsimd.affine_select(
            out=stepb,
            in_=stepb,
            compare_op=mybir.AluOpType.is_ge,
            fill=0.0,
            base=-1,
            pattern=[[-1, P]],
            channel_multiplier=1,
        )
        # stepx[p, c] = -BIG iff p + 128 > c, c in [0, 256): mask for the
        # second s-tile of a diagonal pair (prefix cols fully masked +
        # triangle), full-range so the accumulation group stays uniform
        stepx = const.tile([P, 2 * P], bf16, name="stepx")
        nc.vector.memset(stepx, -BIG)
        nc.gpsimd.affine_select(
            out=stepx,
            in_=stepx,
            compare_op=mybir.AluOpType.is_ge,
            fill=0.0,
            base=P - 1,
            pattern=[[-1, 2 * P]],
            channel_multiplier=1,
        )

        # weight staging (f32 DMA) + bf16 casts; [Wk|Wq] packed so one
        # M=128 matmul yields k on partitions 0-63 and q on 64-127
        wkq_st = const.tile([P, CT, 2 * H], f32, name="wkq_st")
        wv_st = const.tile([P, CT, H], f32, name="wv_st")
        wkq_bf = const.tile([P, CT, 2 * H], bf16, name="wkq_bf")
        wv_bf = const.tile([P, CT, H], bf16, name="wv_bf")

        xT = persist.tile([P, CT, T], bf16, name="xT")
        kT = persist.tile([H, T], bf16, name="kT")
        qT = persist.tile([H, T], bf16, name="qT")
        v1 = persist.tile([P, TT, H1], bf16, name="v1")
        out_sb = persist.tile([P, TT, H], f32, name="out_sb")

        nc.vector.memset(v1[:, :, H : H + 1], 1.0)

        from collections import deque

        pending = deque()

        def drain(n):
            while n > 0 and pending:
                pending.popleft()()
                n -= 1

        po_tiles = {}
        # PV matmuls are deferred two items behind their S matmuls so PE has
        # S work to chew on while the exp of the current item runs on ACT
        deferred_pv = deque()

        def flush_pv(upto_bidx=None):
            while deferred_pv and (
                upto_bidx is None
                or deferred_pv[0][0] <= upto_bidx
                or len(deferred_pv) > 2
            ):
                deferred_pv.popleft()[1]()

        def make_item(bidx, pr):
            # pair pr of block bidx: s-tiles 2*pr, 2*pr+1 over the block's
            # 256 columns; pr == bidx is the diagonal pair
            t0 = bidx * CW
            diag = pr == bidx

            def emit():
                mark(f'item b{bidx} p{pr}')
                if pr == 0:
                    po_tiles[bidx] = psO.tile([H1, CW], f32, name="po")
                po = po_tiles[bidx]
                ps_s = psS.tile([P, 2, CW], f32, name="ps_s", tag="s")
                for j in range(2):
                    st = 2 * pr + j
                    nc.tensor.matmul(
                        ps_s[:, j, :],
                        kT[:, st * P : (st + 1) * P],
                        qT[:, t0 : t0 + CW],
                        start=True,
                        stop=not diag,
                    )
                    if diag and j == 0:
                        nc.tensor.matmul(
                            ps_s[:, 0, 0:P],
                            identb,
                            stepb,
                            start=False,
                            stop=True,
                            skip_group_check=True,
                        )
                    elif diag:
                        nc.tensor.matmul(
                            ps_s[:, 1, :],
                            identb,
                            stepx,
                            start=False,
                            stop=True,
                        )
                pt = pt_pool.tile([P, 2, CW], bf16, name="pt")
                nc.scalar.activation(pt, ps_s, Exp, scale=scale)

                def emit_pv():
                    mark(f'pv b{bidx} p{pr}')
                    for j in range(2):
                        st = 2 * pr + j
                        nc.tensor.matmul(
                            po,
                            v1[:, st, :],
                            pt[:, j, :],
                            start=(st == 0),
                            stop=(st == 2 * bidx + 1),
                        )

                deferred_pv.append((bidx, emit_pv))
                if len(deferred_pv) > 2:
                    deferred_pv.popleft()[1]()

            return emit

        def make_out_task(bidx):
            t0 = bidx * CW

            def emit():
                flush_pv(upto_bidx=bidx)
                mark(f'out b{bidx}')
                po = po_tiles[bidx]
                oT = oT_pool.tile([H1, CW], f32, name="oT")
                nc.vector.tensor_copy(out=oT, in_=po)
                # pe shares the chunk ring: its readers (rec/muls) may lag
                # in the DVE queue, which must never gate the S-tile ring
                pe = psB.tile([P, 2, H1], f32, name="pe", tag="bo")
                for c4 in range(2):
                    nc.tensor.transpose(
                        pe[:, c4, :],
                        oT[:, c4 * P : (c4 + 1) * P],
                        ident[:H1, :H1],
                    )
                rec = rec_pool.tile([P, 2, 1], f32, name="rec")
                nc.vector.reciprocal(rec, pe[:, :, H : H + 1])
                tb = t0 // P
                for c4 in range(2):
                    nc.vector.tensor_scalar_mul(
                        out_sb[:, tb + c4, :], pe[:, c4, 0:H], rec[:, c4, :]
                    )
                nc.sync.dma_start(
                    out_r[:, tb : tb + 2, :], out_sb[:, tb : tb + 2, :]
                )

            return emit

        for tt in range(TT):
            mark(f'dma t{tt}')
            xa = xa_pool.tile([P, C], f32r, name="xa")
            nc.sync.dma_start(xa, x_r[tt])
            if tt == 1:
                # weights after the first two x tiles: transposes start ASAP,
                # kq weights first (kq(0) is strip-critical), each half cast
                # on Pool right as its DMA lands
                nc.sync.dma_start(
                    wkq_st[:, :, 0:H], wk_d.rearrange("(o p) h -> p o h", p=P)
                )
                nc.gpsimd.tensor_copy(
                    out=wkq_bf[:, :, 0:H], in_=wkq_st[:, :, 0:H]
                )
                nc.sync.dma_start(
                    wkq_st[:, :, H : 2 * H],
                    wq_d.rearrange("(o p) h -> p o h", p=P),
                )
                nc.gpsimd.tensor_copy(
                    out=wkq_bf[:, :, H : 2 * H], in_=wkq_st[:, :, H : 2 * H]
                )
                nc.sync.dma_start(
                    wv_st, wv_d.rearrange("(o p) h -> p o h", p=P)
                )
                nc.gpsimd.tensor_copy(out=wv_bf, in_=wv_st)

            # Phase A: transpose this t-tile into xT (bf16), attention items
            # drained between transpose groups
            tsl = slice(tt * P, (tt + 1) * P)
            for g in range(2):
                mark(f'T t{tt} g{g}')
                ps_t = psA.tile([P, 4, P], f32r, name="ps_t", tag="av")
                for i in range(4):
                    ci = 4 * g + i
                    nc.tensor.transpose(
                        ps_t[:, i, :], xa[:, ci * P : (ci + 1) * P], identr
                    )
                dst = xT[:, 4 * g : 4 * g + 4, tsl]
                # early tiles: ACT takes half the copies (its exp stream
                # hasn't started); later tiles: all on DVE
                if tt < 8 and g == 1:
                    nc.scalar.copy(out=dst, in_=ps_t)
                else:
                    nc.vector.tensor_copy(out=dst, in_=ps_t)
                drain(1 if tt < 8 else 2)

            if tt % 2 == 0:
                continue
            k = tt // 2  # completed 256-col block

            # Phase B-kq for this block's 256 columns (strip-critical: the
            # kT/qT copies unblock this block's attention items)
            mark(f'KQ k{k}')
            csl = slice(k * CW, (k + 1) * CW)
            pkq = psB.tile([P, CW], f32, name="pkq", tag="bo")
            for ci in range(CT):
                nc.tensor.matmul(
                    pkq,
                    wkq_bf[:, ci, :],
                    xT[:, ci, csl],
                    start=(ci == 0),
                    stop=(ci == CT - 1),
                )
            nc.vector.tensor_copy(out=kT[:, csl], in_=pkq[0:H, :])
            # partition-shift copy 64-127 -> 0-63 (legal on DVE)
            nc.vector.tensor_copy(out=qT[:, csl], in_=pkq[H:P, :])
            # out-task(k) is held back one block: its oT copy must not enter
            # the in-order DVE queue while strip k is still running, or it
            # parks DVE and starves the next chunk's xT copies behind it
            pending.extend(make_item(k, pr) for pr in range(k + 1))
            if k > 0:
                pending.append(make_out_task(k - 1))
            drain(2)

            # Phase B-v: v in natural [t, h] layout for this block's 2 tiles
            mark(f'V k{k}')
            psv = psA.tile([P, 2, H], f32, name="psv", tag="av")
            for j in range(2):
                vsl = slice((2 * k + j) * P, (2 * k + j + 1) * P)
                for ci in range(CT):
                    nc.tensor.matmul(
                        psv[:, j, :],
                        xT[:, ci, vsl],
                        wv_bf[:, ci, :],
                        start=(ci == 0),
                        stop=(ci == CT - 1),
                    )
            nc.vector.tensor_copy(out=v1[:, 2 * k : 2 * k + 2, 0:H], in_=psv)
            drain(2)

        pending.append(make_out_task(NBLK - 1))
        drain(len(pending))
        flush_pv()


_NC_CACHE = {}


def build_nc():
    if "nc" in _NC_CACHE:
        return _NC_CACHE["nc"]
    _patch_drain_split()
    f32r = mybir.dt.float32r
    f32 = mybir.dt.float32
    nc = bass.Bass(
        "TRN2", target_bir_lowering=False, debug=False, num_devices=N_CORES
    )
    x_d = nc.dram_tensor("x", [T, C], f32r, kind="ExternalInput").ap()
    wk_d = nc.dram_tensor("Wk", [C, H], f32, kind="ExternalInput").ap()
    wq_d = nc.dram_tensor("Wq", [C, H], f32, kind="ExternalInput").ap()
    wv_d = nc.dram_tensor("Wv", [C, H], f32, kind="ExternalInput").ap()
    out_d = nc.dram_tensor("out", [T, H], f32, kind="ExternalOutput").ap()
    with tile.TileContext(nc) as tc:
        _emit(tc, out_d, x_d, wk_d, wq_d, wv_d)
    _NC_CACHE["nc"] = nc
    return nc


def kernel(x, Wk, Wq, Wv, **run_kwargs):
    """Full-input entry point: shard over batch, run on cores 0-7, gather."""
    x = np.ascontiguousarray(np.asarray(x), dtype=np.float32)
    Wk = np.ascontiguousarray(np.asarray(Wk), dtype=np.float32)
    Wq = np.ascontiguousarray(np.asarray(Wq), dtype=np.float32)
    Wv = np.ascontiguousarray(np.asarray(Wv), dtype=np.float32)
    assert x.shape == (B, T, C), x.shape

    nc = build_nc()
    in_maps = [
        {"x": np.ascontiguousarray(x[b]), "Wk": Wk, "Wq": Wq, "Wv": Wv}
        for b in range(B)
    ]
    res = bass_utils.run_bass_kernel_spmd(
        nc, in_maps, core_ids=list(range(N_CORES)), **run_kwargs
    )
    out = np.stack([res.results[b]["out"] for b in range(B)], axis=0)
    if run_kwargs:
        kernel.last_results = res
    return out.astype(np.float32)
